# revision 28
# baseline (speedup 1.0000x reference)
"""DT4Rec dense transformer forward on 8 Trainium2 NeuronCores.

Data-parallel over batch: core c computes sequences [2c, 2c+1] of the
B=16 batch through the full 6-layer decision-transformer, in fp32
(float32r matmuls, ~1 cyc/row).  All activations are kept feature-major
(F-layout: [feature partitions x tokens free]) so every weight matmul
uses the weight in its natural [d_in, d_out] layout as the stationary
operand and no on-device transposes are needed anywhere:

  - Q/K:   out[d_out, t]  = Wq[d_in, d_out].T @ h[d_in, t]
  - V:     out[t, d_out]  = h[d_in, t].T @ Wv[d_in, d_out]   (token-major)
  - S^T:   out[j, i]      = k_h[d, j].T @ q_h[d, i]          (pre-transposed
           scores so softmax-normalized A^T feeds A@V directly)
  - soft:  exp (no max-sub; logits are O(1)), causal+pad mask as 0/1
           multiply, denominators via ones-matmul over partitions,
           normalization via K=1 broadcast matmuls
  - LN:    stats via ones-matmuls over the feature (partition) dim;
           gamma/beta folded into the consumer weights host-side.

y_len is accepted and unused (matches the reference, which derives
sequence length from states.shape).
"""

import os
import sys

import ml_dtypes
import numpy as np

for _p in ("/opt/trn_rl_repo", "/root/.axon_site/_ro/trn_rl_repo"):
    if os.path.isdir(_p) and _p not in sys.path:
        sys.path.append(_p)

import concourse.bass as bass  # noqa: E402
import concourse.tile as tile  # noqa: E402
from concourse import bacc, mybir  # noqa: E402
from concourse.bass import ts  # noqa: E402
from concourse.bass_utils import run_bass_kernel_spmd  # noqa: E402

F32 = mybir.dt.float32
F32R = mybir.dt.float32r
BF16 = mybir.dt.bfloat16
# dtype for streamed weights + matmul activations (residual/LN/softmax
# normalization stay fp32)
F16 = mybir.dt.float16
WDT = F16

D = 1024
H = 16
HD = 64
NL = 6
L = 64
TN = 3 * L          # 192 tokens per sequence
NSEQ = 2            # sequences per core
NT = NSEQ * TN      # 384 tokens per core
DF = 4 * D
NB = 100
NCORES = 8
DEBUG = False
SCALE = 1.0 / np.sqrt(HD)
EPS = 1e-5

# token chunks per sequence: (chunk_idx, vt_tile_idx, jcnt)
# Vt tiles hold tokens [0:128],[128:192],[192:320],[320:384]
_CHUNKS = {0: [(0, 0, 128), (1, 1, 64)], 1: [(0, 2, 128), (1, 3, 64)]}
_VT_SPANS = [(0, 128), (128, 64), (192, 128), (320, 64)]  # (tok0, tcnt)

_PARAMS = [
    # per-core activations
    ("states_f", [91, 2 * L], F32R),
    ("act_f", [27, 2 * L], F32R),
    ("rtg_row", [1, 2 * L], F32R),
    ("mask_t", [128, 4 * TN], WDT),       # blocks (chunk, seq): cs = c*2+s
    ("pos_f", [128, 8 * NT], F32),
    # constants
    ("ones_col", [128, 1], F32R),
    ("ones_col_w", [128, 1], WDT),
        ("ones_row", [1, 128], F32R),
    ("sel_a", [1, 128], F32R),            # 1 at cols 0..63
    ("sel_b", [1, 128], F32R),            # 1 at cols 64..127
    # encoders
    ("se_w1", [91, D], F32R),
    ("se_b1", [128, 8], F32),
    ("se_w2", [D, D], WDT),
    ("se_b2", [128, 8], F32),
    ("ae_w1", [27, D], F32R),
    ("ae_b1", [128, 8], F32),
    ("ae_w2", [D, D], WDT),
    ("ae_b2", [128, 8], F32),
    # autodis
    ("bucket_col", [1, NB], F32R),
    ("adret_row", [1, NB], F32R),
    ("adres_m", [NB, NB], F32R),
    ("adbw_row", [1, D], F32R),
    ("adb_col", [128, 8], F32),
    # transformer blocks (ln1 folded into Wq/Wk/Wv, ln2 into Wm1)
    ("Wq_a", [NL, D, D], WDT),
    ("bq_a", [NL, 128, 8], F32),
    ("Wk_a", [NL, D, D], WDT),
    ("bk_a", [NL, 128, 8], F32),
    ("Wv_a", [NL, D, D], WDT),
    ("bv_row_a", [NL, 1, D], F32R),
    ("Wp_a", [NL, D, D], WDT),
    ("bp_a", [NL, 128, 8], F32),
    ("Wm1_a", [NL, D, DF], WDT),
    ("bm1_a", [NL, 128, 32], F32),
    ("Wm2_a", [NL, DF, D], WDT),
    ("bm2_a", [NL, 128, 8], F32),
    # head
    ("lin_w_m", [D, 27], F32R),
    ("lin_b_col", [27, 1], F32),
]


def _emit(nc, tc, P, out_h):
    def dump(name, ap):
        if not DEBUG:
            return
        t = nc.dram_tensor(name, list(ap.shape), ap.dtype, kind="ExternalOutput")
        nc.sync.dma_start(out=t[:], in_=ap)

    Add = mybir.AluOpType.add
    Sub = mybir.AluOpType.subtract
    Mult = mybir.AluOpType.mult
    AF = mybir.ActivationFunctionType

    consts = tc.alloc_tile_pool(name="consts", bufs=1)
    persist = tc.alloc_tile_pool(name="persist", bufs=1)
    work = tc.alloc_tile_pool(name="work", bufs=2)
    wstream = tc.alloc_tile_pool(name="wstream", bufs=4)
    pp = tc.alloc_tile_pool(name="pp", bufs=2, space="PSUM")

    def cload(name, shape=None, dtype=None, src=None):
        ph = P[name]
        shape = shape or list(ph.shape)
        t = consts.tile(shape, dtype or ph.dtype, tag=name)
        nc.sync.dma_start(out=t[:], in_=src if src is not None else ph[:])
        return t

    # ---- constants / small weights resident in SBUF ----
    ones_col = cload("ones_col")
    ones_col_w = cload("ones_col_w")
    ones_row = cload("ones_row")
    sel_a = cload("sel_a")
    sel_b = cload("sel_b")
    mask_sb = cload("mask_t")
    se_w1 = cload("se_w1")
    se_b1 = cload("se_b1")
    se_b2 = cload("se_b2")
    ae_w1 = cload("ae_w1")
    ae_b1 = cload("ae_b1")
    ae_b2 = cload("ae_b2")
    bucket_col = cload("bucket_col")
    adret_row = cload("adret_row")
    adres_m = cload("adres_m")
    adbw_row = cload("adbw_row")
    adb_col = cload("adb_col")
    states_sb = cload("states_f")
    act_sb = cload("act_f")
    rtg_sb = cload("rtg_row")
    lin_b = cload("lin_b_col")
    eps_col = consts.tile([128, 1], F32, tag="epsc")
    nc.vector.memset(eps_col[:], EPS)
    dummy_row = consts.tile([1, 1], F32, tag="dummy")
    nc.vector.memset(dummy_row[:], 1.0)
    dummy_row2 = consts.tile([1, 1], F32, tag="dummy2")
    nc.vector.memset(dummy_row2[:], 1.0)
    # all-layer bias tables: [NL,128,C] -> [128, NL, C]
    def bload(name, C):
        t = consts.tile([128, NL, C], F32, tag=name)
        nc.sync.dma_start(
            out=t[:], in_=P[name][:].rearrange("l p c -> p l c")
        )
        return t

    bq_sb = bload("bq_a", 8)
    bk_sb = bload("bk_a", 8)
    bp_sb = bload("bp_a", 8)
    bm2_sb = bload("bm2_a", 8)
    bm1_sb = bload("bm1_a", 32)
    linw_sb = consts.tile([128, 8, 27], F32R, tag="lin_w_m")
    nc.sync.dma_start(
        out=linw_sb[:], in_=P["lin_w_m"][:].rearrange("(t p) m -> p t m", p=128)
    )

    # ---- persistent activation tiles ----
    x_sb = persist.tile([128, 8 * NT], F32R, tag="x")
    h_sb_pool = work  # h allocated per use, tag-shared
    q_sb = persist.tile([128, 8 * NT], WDT, tag="q")
    k_sb = persist.tile([128, 8 * NT], WDT, tag="k")
    y_sb = persist.tile([128, 8 * NT], WDT, tag="y")
    gelu_sb = persist.tile([128, 32 * NT], WDT, tag="gelu")
    vt_sb = [
        persist.tile([tcnt, 2 * 512], WDT, tag=f"vt{i}", name=f"vt{i}")
        for i, (_, tcnt) in enumerate(_VT_SPANS)
    ]

    xv = x_sb[:].rearrange("p (dt s l r) -> p dt s l r", dt=8, s=2, r=3)

    # =====================  embeddings  =====================
    with tc.tile_pool(name="enc", bufs=2) as enc:

        def enc_stage(w1_sb, kdim, rhs, b1_sb, w2_param, b2_sb, func2, r_idx):
            # L1: e1 = gelu(rhs.T @ W1 + b1)   [resident small W1]
            e1 = enc.tile([128, 8 * 128], WDT, tag="e1", bufs=1, name="e1")
            for dout in range(8):
                ps = pp.tile([128, 2 * L], F32, tag="acc", bufs=4, name="enc_ps")
                nc.tensor.matmul(
                    ps[:], w1_sb[0:kdim, ts(dout, 128)], rhs[0:kdim, :],
                    start=True, stop=True,
                )
                nc.scalar.activation(
                    e1[:, ts(dout, 128)], ps[:], AF.Gelu,
                    bias=b1_sb[:, dout : dout + 1],
                )
            # L2: stream [128,4,512] chunks of W2
            for dg in range(2):
                ps_l = []
                for kt0 in range(0, 8, 4):
                    wch = wstream.tile(
                        [128, 4, 512], WDT, tag="wch16", bufs=8, name="wche"
                    )
                    nc.sync.dma_start(
                        out=wch[:],
                        in_=w2_param[
                            kt0 * 128 : (kt0 + 4) * 128,
                            dg * 512 : (dg + 1) * 512,
                        ].rearrange("(g p) c -> p g c", p=128),
                    )
                    for g in range(4):
                        kt = kt0 + g
                        for j in range(4):
                            if kt == 0:
                                ps_l.append(
                                    pp.tile([128, 2 * L], F32, tag="acc", bufs=4,
                                            name="enc_acc")
                                )
                            nc.tensor.matmul(
                                ps_l[j][:], wch[:, g, ts(j, 128)],
                                e1[:, ts(kt, 128)],
                                start=(kt == 0), stop=(kt == 7),
                            )
                for j in range(4):
                    dout = dg * 4 + j
                    for s in range(2):
                        nc.scalar.activation(
                            xv[:, dout, s, :, r_idx],
                            ps_l[j][:, s * L : (s + 1) * L],
                            func2,
                            bias=b2_sb[:, dout : dout + 1],
                        )

        enc_stage(se_w1, 91, states_sb, se_b1, P["se_w2"], se_b2, AF.Gelu, 1)
        enc_stage(ae_w1, 27, act_sb, ae_b1, P["ae_w2"], ae_b2, AF.Identity, 2)

        # ---- Autodis rtg embedding -> slot 0 ----
        meta_sb = enc.tile([NB, D], F32R, tag="meta", bufs=1)
        for nch in range(2):
            mps = pp.tile([NB, 512], F32, tag="acc", bufs=4, name="mps")
            nc.tensor.matmul(
                mps[:], bucket_col[:], adbw_row[0:1, nch * 512 : (nch + 1) * 512],
                start=True, stop=True,
            )
            nc.scalar.copy(meta_sb[:, nch * 512 : (nch + 1) * 512], mps[:])
        sps = pp.tile([NB, 2 * L], F32, tag="acc", bufs=4, name="sps")
        nc.tensor.matmul(sps[:], adret_row[:], rtg_sb[:], start=True, stop=True)
        s1_sb = enc.tile([NB, 2 * L], F32R, tag="s1", bufs=1)
        nc.scalar.activation(s1_sb[:], sps[:], AF.Lrelu, alpha=0.01)
        s2ps = pp.tile([NB, 2 * L], F32, tag="acc", bufs=4, name="s2ps")
        nc.tensor.matmul(s2ps[:], adres_m[:], s1_sb[:], start=True, stop=True)
        s2_sb = enc.tile([NB, 2 * L], F32R, tag="s2", bufs=1)
        nc.vector.tensor_tensor(s2_sb[:], s2ps[:], s1_sb[:], op=Add)
        p_sb = enc.tile([NB, 2 * L], F32R, tag="pexp", bufs=1)
        nc.scalar.activation(p_sb[:], s2_sb[:], AF.Exp)
        dnp = pp.tile([1, 2 * L], F32, tag="acc", bufs=4, name="dnp")
        nc.tensor.matmul(dnp[:], ones_col[0:NB, :], p_sb[:], start=True, stop=True)
        dn_sb = enc.tile([1, 2 * L], F32, tag="dn", bufs=1)
        nc.vector.tensor_scalar(dn_sb[:], dnp[:], 1e-8, None, op0=Add)
        rec_sb = enc.tile([1, 2 * L], F32, tag="rec", bufs=1)
        nc.vector.reciprocal(rec_sb[:], dn_sb[:])
        rb_sb = enc.tile([128, 2 * L], F32, tag="rb", bufs=1)
        nc.gpsimd.partition_broadcast(rb_sb[:], rec_sb[0:1, :])
        for dout in range(8):
            eps_ = pp.tile([128, 2 * L], F32, tag="acc", bufs=4, name="eps_")
            nc.tensor.matmul(
                eps_[:], meta_sb[:, ts(dout, 128)], p_sb[:], start=True, stop=True
            )
            for s in range(2):
                tmp = enc.tile([128, L], F32, tag="rtmp", bufs=3)
                nc.vector.tensor_tensor(
                    tmp[:], eps_[:, s * L : (s + 1) * L],
                    rb_sb[:, s * L : (s + 1) * L], op=Mult,
                )
                nc.vector.tensor_scalar(
                    xv[:, dout, s, :, 0], tmp[:],
                    adb_col[:, dout : dout + 1], None, op0=Add,
                )

        # ---- add positional embedding ----
        for dt in range(8):
            pt = enc.tile([128, NT], F32, tag="post", bufs=2, name="pt")
            nc.sync.dma_start(out=pt[:], in_=P["pos_f"][:, ts(dt, NT)])
            nc.vector.tensor_tensor(
                x_sb[:, ts(dt, NT)], x_sb[:, ts(dt, NT)], pt[:], op=Add
            )

    dump("d_x0", x_sb[:])

    mview = mask_sb[:].rearrange("p (cs i) -> p cs i", cs=4)

    # =====================  transformer layers  =====================
    def layernorm_to(h_t, lid):
        """h = (x - mu(x)) * rstd(x), feature dim = partitions."""
        mu_ps = pp.tile([1, NT], F32, tag="acc", bufs=4, name="mu_ps")
        s2_ps = pp.tile([1, NT], F32, tag="acc", bufs=4, name="s2_ps")
        for dt in range(8):
            sq = work.tile([128, NT], F32R, tag="scratch", bufs=2, name="sq")
            nc.scalar.square(sq[:], x_sb[:, ts(dt, NT)])
            nc.tensor.matmul(
                mu_ps[:], ones_col[:], x_sb[:, ts(dt, NT)],
                start=(dt == 0), stop=(dt == 7),
            )
            nc.tensor.matmul(
                s2_ps[:], ones_col[:], sq[:],
                start=(dt == 0), stop=(dt == 7),
            )
        mu_row = work.tile([1, NT], F32, tag="rows", bufs=4)
        nc.vector.tensor_scalar(mu_row[:], mu_ps[:], 1.0 / D, None, op0=Mult)
        m2 = work.tile([1, NT], F32, tag="rows", bufs=4)
        nc.vector.tensor_tensor(m2[:], mu_row[:], mu_row[:], op=Mult)
        v0 = work.tile([1, NT], F32, tag="rows", bufs=4)
        nc.vector.scalar_tensor_tensor(
            v0[:], s2_ps[:], 1.0 / D, m2[:], op0=Mult, op1=Sub
        )
        # rstd = (v+eps)^-0.5 = exp(-0.5*ln(v+eps)) — Ln/Exp share the
        # ACT table set the kernel already uses, avoiding the ~1.4us
        # func-table reload that Sqrt triggers on the critical path.
        lnv = work.tile([1, NT], F32, tag="rows", bufs=4)
        nc.scalar.activation(
            lnv[:], v0[:], mybir.ActivationFunctionType.Ln, bias=eps_col[0:1, :]
        )
        rstd = work.tile([1, NT], F32, tag="rows", bufs=4)
        nc.scalar.activation(
            rstd[:], lnv[:], mybir.ActivationFunctionType.Exp, scale=-0.5
        )
        mub = work.tile([128, NT], F32, tag="bcast", bufs=2, name="mub")
        nc.gpsimd.partition_broadcast(mub[:], mu_row[0:1, :])
        rstdb = work.tile([128, NT], F32, tag="bcast", bufs=2, name="rstdb")
        nc.gpsimd.partition_broadcast(rstdb[:], rstd[0:1, :])
        for dt in range(8):
            td = work.tile([128, NT], F32, tag="scratch", bufs=2, name="td")
            nc.vector.tensor_tensor(td[:], x_sb[:, ts(dt, NT)], mub[:], op=Sub)
            nc.vector.tensor_tensor(h_t[:, ts(dt, NT)], td[:], rstdb[:], op=Mult)

    def proj_F(w_layer, rhs_fn, out_fn, n_kt, m_total):
        KG = 4  # kt-tiles fetched per DMA (512KB bf16 transfers)
        for dg in range(m_total // 512):
            ps_l = []
            for kt0 in range(0, n_kt, KG):
                wch = wstream.tile(
                    [128, KG, 512], WDT, tag="wch16", bufs=8, name="wch"
                )
                nc.sync.dma_start(
                    out=wch[:],
                    in_=w_layer[
                        kt0 * 128 : (kt0 + KG) * 128, dg * 512 : (dg + 1) * 512
                    ].rearrange("(g p) c -> p g c", p=128),
                )
                for g in range(KG):
                    kt = kt0 + g
                    for j in range(4):
                        if kt == 0:
                            ps_l.append(
                                pp.tile([128, NT], F32, tag="acc", bufs=4,
                                        name="acc_t")
                            )
                        nc.tensor.matmul(
                            ps_l[j][:], wch[:, g, ts(j, 128)], rhs_fn(kt),
                            start=(kt == 0), stop=(kt == n_kt - 1),
                        )
            for j in range(4):
                out_fn(dg * 4 + j, ps_l[j])

    AF = mybir.ActivationFunctionType

    nc.scalar.activation(dummy_row2[:], dummy_row[:], AF.Ln)
    for l in range(NL):
        # ---------- LN1 -> h ----------
        h_t = work.tile([128, 8 * NT], WDT, tag="h", bufs=1)
        layernorm_to(h_t, l)
        h_rhs = lambda kt, _h=h_t: _h[:, ts(kt, NT)]

        # ---------- Q, K ----------
        def q_out(dout, ps, _b=bq_sb, _t=q_sb):
            nc.vector.tensor_scalar(
                _t[:, ts(dout, NT)], ps[:], _b[:, l, dout : dout + 1], None, op0=Add
            )

        def k_out(dout, ps, _b=bk_sb, _t=k_sb):
            nc.vector.tensor_scalar(
                _t[:, ts(dout, NT)], ps[:], _b[:, l, dout : dout + 1], None, op0=Add
            )

        if l == 0:
            dump("d_h0", h_t[:])
        proj_F(P["Wq_a"][l], h_rhs, q_out, 8, D)
        proj_F(P["Wk_a"][l], h_rhs, k_out, 8, D)
        if l == 0:
            dump("d_q0", q_sb[:])
            dump("d_k0", k_sb[:])

        # ---------- V (token-major, bias via K=1 matmul) ----------
        bvrow = wstream.tile([1, D], F32R, tag="bvrow", bufs=2, name="bvrow")
        nc.sync.dma_start(out=bvrow[:], in_=P["bv_row_a"][l])
        for nch in range(2):
            ps_m = []
            for kt0 in range(0, 8, 4):
                wch = wstream.tile(
                    [128, 4, 512], WDT, tag="wch16", bufs=8, name="wchv"
                )
                nc.sync.dma_start(
                    out=wch[:],
                    in_=P["Wv_a"][l][
                        kt0 * 128 : (kt0 + 4) * 128, nch * 512 : (nch + 1) * 512
                    ].rearrange("(g p) c -> p g c", p=128),
                )
                for g in range(4):
                    kt = kt0 + g
                    for m, (tok0, tcnt) in enumerate(_VT_SPANS):
                        if kt == 0:
                            ps_m.append(
                                pp.tile([128, 512], F32, tag="acc", bufs=4,
                                        name="vacc_t")
                            )
                        nc.tensor.matmul(
                            ps_m[m][0:tcnt, :],
                            h_t[:, kt * NT + tok0 : kt * NT + tok0 + tcnt],
                            wch[:, g, :],
                            start=(kt == 0), stop=False,
                        )
            for m, (tok0, tcnt) in enumerate(_VT_SPANS):
                nc.tensor.matmul(
                    ps_m[m][0:tcnt, :],
                    ones_row[0:1, 0:tcnt],
                    bvrow[0:1, nch * 512 : (nch + 1) * 512],
                    start=False, stop=True,
                )
                nc.scalar.copy(
                    vt_sb[m][0:tcnt, nch * 512 : (nch + 1) * 512], ps_m[m][0:tcnt, :]
                )

        # ---------- attention ----------
        qv = q_sb[:].rearrange("p (dt s i) -> p dt s i", dt=8, s=2)
        kv = k_sb[:].rearrange("p (dt s i) -> p dt s i", dt=8, s=2)
        for s in range(2):
            a_t = [
                work.tile([128, H * TN], WDT, tag="a0", bufs=1, name="a0"),
                work.tile([64, H * TN], WDT, tag="a1", bufs=1, name="a1"),
            ]
            for c, vtix, jcnt in _CHUNKS[s]:
                joff = 0 if c == 0 else 128
                for h in range(H):
                    dt, hp = h // 2, h % 2
                    st_ps = pp.tile(
                        [128, TN], F32, tag="big", bufs=2, name="st_ps"
                    )
                    nc.tensor.matmul(
                        st_ps[0:jcnt, :],
                        kv[hp * 64 : (hp + 1) * 64, dt, s, joff : joff + jcnt],
                        qv[hp * 64 : (hp + 1) * 64, dt, s, :],
                        start=True, stop=True,
                    )
                    nc.scalar.activation(
                        a_t[c][0:jcnt, ts(h, TN)], st_ps[0:jcnt, :], AF.Exp,
                        scale=SCALE,
                    )
                    nc.vector.tensor_tensor(
                        a_t[c][0:jcnt, ts(h, TN)], a_t[c][0:jcnt, ts(h, TN)],
                        mview[0:jcnt, c * 2 + s, :], op=Mult,
                    )
            if l == 0 and s == 0:
                dump("d_a00", a_t[0][:])
                dump("d_a01", a_t[1][:])
            # denominators -> reciprocal row [1, H*TN]
            r_row = work.tile([1, H * TN], F32R, tag="rrow", bufs=1)
            for b6 in range(6):
                dn_ps = pp.tile([1, 512], F32, tag="acc", bufs=4, name="dn_ps")
                nc.tensor.matmul(
                    dn_ps[:], ones_col_w[0:128, :], a_t[0][:, ts(b6, 512)],
                    start=True, stop=False,
                )
                nc.tensor.matmul(
                    dn_ps[:], ones_col_w[0:64, :], a_t[1][0:64, ts(b6, 512)],
                    start=False, stop=True,
                )
                nc.vector.reciprocal(r_row[0:1, ts(b6, 512)], dn_ps[:])
            # y = (V @ A^T) * Nrm ; per-head PSUM at base 0 (f32r matmul
            # cannot target PSUM base-partition 64), ACT copy shifts parts.
            for dt in range(8):
                yh_ps = [
                    pp.tile([64, TN], F32, tag="acc", bufs=4, name="yh_ps")
                    for _ in range(2)
                ]
                for hp in range(2):
                    h = dt * 2 + hp
                    for ci, (c, vtix, jcnt) in enumerate(_CHUNKS[s]):
                        nc.tensor.matmul(
                            yh_ps[hp][:],
                            vt_sb[vtix][0:jcnt, ts(h, 64)],
                            a_t[c][0:jcnt, ts(h, TN)],
                            start=(ci == 0), stop=(ci == 1),
                        )
                nrm_ps = pp.tile([128, TN], F32, tag="acc", bufs=4, name="nrm_ps")
                nc.tensor.matmul(
                    nrm_ps[:], sel_a[:], r_row[0:1, ts(dt * 2, TN)],
                    start=True, stop=False,
                )
                nc.tensor.matmul(
                    nrm_ps[:], sel_b[:], r_row[0:1, ts(dt * 2 + 1, TN)],
                    start=False, stop=True,
                )
                ysl = y_sb[:, dt * NT + s * TN : dt * NT + (s + 1) * TN]
                nc.scalar.copy(ysl[0:64, :], yh_ps[0][:])
                nc.scalar.copy(ysl[64:128, :], yh_ps[1][:])
                nc.vector.tensor_tensor(ysl, ysl, nrm_ps[:], op=Mult)

        if l == 0:
            dump("d_y0", y_sb[:])
        # ---------- attention proj + residual ----------
        def p_out(dout, ps, _b=bp_sb):
            t = work.tile([128, NT], F32, tag="rtmp2", bufs=2)
            nc.scalar.activation(
                t[:], ps[:], AF.Identity, bias=_b[:, l, dout : dout + 1]
            )
            nc.vector.tensor_tensor(
                x_sb[:, ts(dout, NT)], x_sb[:, ts(dout, NT)], t[:], op=Add
            )

        proj_F(P["Wp_a"][l], lambda kt: y_sb[:, ts(kt, NT)], p_out, 8, D)

        if l == 0:
            dump("d_xattn0", x_sb[:])
        # ---------- LN2 -> h2 ----------
        h2_t = work.tile([128, 8 * NT], WDT, tag="h", bufs=1)
        layernorm_to(h2_t, l)
        # prefetch the gelu func-table while fc1 matmuls accumulate
        nc.scalar.activation(
            dummy_row2[:], dummy_row[:], AF.Gelu
        )

        # ---------- MLP ----------
        def fc1_out(dout, ps, _b=bm1_sb):
            nc.scalar.activation(
                gelu_sb[:, ts(dout, NT)], ps[:], AF.Gelu,
                bias=_b[:, l, dout : dout + 1],
            )

        proj_F(P["Wm1_a"][l], lambda kt, _h=h2_t: _h[:, ts(kt, NT)], fc1_out, 8, DF)

        def fc2_out(dout, ps, _b=bm2_sb):
            t = work.tile([128, NT], F32, tag="rtmp2", bufs=2)
            nc.scalar.activation(
                t[:], ps[:], AF.Identity, bias=_b[:, l, dout : dout + 1]
            )
            nc.vector.tensor_tensor(
                x_sb[:, ts(dout, NT)], x_sb[:, ts(dout, NT)], t[:], op=Add
            )

        # prefetch the ln/exp func-table (next LN1) while fc2 runs
        nc.scalar.activation(
            dummy_row2[:], dummy_row[:], AF.Ln
        )
        proj_F(P["Wm2_a"][l], lambda kt: gelu_sb[:, ts(kt, NT)], fc2_out, 32, D)

    dump("d_xfin", x_sb[:])
    # =====================  head  =====================
    o_ps = pp.tile([27, NT], F32, tag="acc", bufs=4, name="o_ps")
    for kt in range(8):
        nc.tensor.matmul(
            o_ps[:], linw_sb[:, kt, :], x_sb[:, ts(kt, NT)],
            start=(kt == 0), stop=(kt == 7),
        )
    o_sb = work.tile([27, NT], F32, tag="rtmp2", bufs=2, name="osb")
    nc.vector.tensor_scalar(o_sb[:], o_ps[:], lin_b[0:27, 0:1], None, op0=Add)
    nc.sync.dma_start(out=out_h[:], in_=o_sb[:])

    pp.release()
    wstream.release()
    work.release()
    persist.release()
    consts.release()


def build_program():
    nc = bacc.Bacc()
    P = {n: nc.declare_dram_parameter(n, s, d, isOutput=False) for n, s, d in _PARAMS}
    out_h = nc.declare_dram_parameter("out_f", [27, NT], F32, isOutput=True)
    with tile.TileContext(nc) as tc, nc.allow_low_precision(
        reason="float32r outputs are fp32-width; rounding is intended"
    ):
        _emit(nc, tc, P, out_h)
    nc.finalize()
    return nc


_NC_CACHE = None


def _get_nc():
    global _NC_CACHE
    if _NC_CACHE is None:
        _NC_CACHE = build_program()
    return _NC_CACHE


def host_prep(inputs):
    """Build the shared (weights) and per-core input arrays."""
    f = lambda a: np.ascontiguousarray(np.asarray(a), dtype=np.float32)
    shared = {}
    shared["ones_col"] = np.ones((128, 1), np.float32)
    shared["ones_col_w"] = np.ones((128, 1), np.float16)
    shared["ones_row"] = np.ones((1, 128), np.float32)
    sa = np.zeros((1, 128), np.float32); sa[0, :64] = 1.0
    sb = np.zeros((1, 128), np.float32); sb[0, 64:] = 1.0
    shared["sel_a"], shared["sel_b"] = sa, sb

    col8 = lambda v: f(v).reshape(8, 128).T.copy()        # [1024] -> [128,8]
    shared["se_w1"] = f(inputs["se_w1"])
    shared["se_b1"] = col8(inputs["se_b1"])
    shared["se_w2"] = f(inputs["se_w2"]).astype(np.float16)
    shared["se_b2"] = col8(inputs["se_b2"])
    shared["ae_w1"] = f(inputs["ae_w1"])
    shared["ae_b1"] = col8(inputs["ae_b1"])
    shared["ae_w2"] = f(inputs["ae_w2"]).astype(np.float16)
    shared["ae_b2"] = col8(inputs["ae_b2"])

    bucket = np.linspace(0.0, 100.0, NB, dtype=np.float64).astype(np.float32)
    shared["bucket_col"] = bucket.reshape(1, NB)
    shared["adret_row"] = f(inputs["ad_ret_w"]).reshape(1, NB)
    shared["adres_m"] = f(inputs["ad_res_w"])
    shared["adbw_row"] = f(inputs["ad_bucket_w"]).reshape(1, D)
    shared["adb_col"] = col8(inputs["ad_bucket_b"])

    g1 = f(inputs["ln1_g"]); b1 = f(inputs["ln1_b"])
    g2 = f(inputs["ln2_g"]); b2 = f(inputs["ln2_b"])
    Wq = f(inputs["Wq"]); Wk = f(inputs["Wk"]); Wv = f(inputs["Wv"])
    Wp = f(inputs["Wp"]); Wm1 = f(inputs["Wm1"]); Wm2 = f(inputs["Wm2"])
    bq = f(inputs["bq"]); bk = f(inputs["bk"]); bv = f(inputs["bv"])
    bp = f(inputs["bp"]); bm1 = f(inputs["bm1"]); bm2 = f(inputs["bm2"])

    Wq_f = g1[:, :, None] * Wq
    Wk_f = g1[:, :, None] * Wk
    Wv_f = g1[:, :, None] * Wv
    Wm1_f = g2[:, :, None] * Wm1
    bq_f = bq + np.einsum("ld,ldo->lo", b1, Wq)
    bk_f = bk + np.einsum("ld,ldo->lo", b1, Wk)
    bv_f = bv + np.einsum("ld,ldo->lo", b1, Wv)
    bm1_f = bm1 + np.einsum("ld,ldo->lo", b2, Wm1)

    colL = lambda v, C: np.ascontiguousarray(
        v.reshape(NL, C, 128).transpose(0, 2, 1)
    )  # [NL, C*128] -> [NL,128,C]
    shared["Wq_a"] = np.ascontiguousarray(Wq_f).astype(np.float16)
    shared["bq_a"] = colL(bq_f, 8)
    shared["Wk_a"] = np.ascontiguousarray(Wk_f).astype(np.float16)
    shared["bk_a"] = colL(bk_f, 8)
    shared["Wv_a"] = np.ascontiguousarray(Wv_f).astype(np.float16)
    shared["bv_row_a"] = np.ascontiguousarray(bv_f.reshape(NL, 1, D))
    shared["Wp_a"] = np.ascontiguousarray(Wp).astype(np.float16)
    shared["bp_a"] = colL(bp, 8)
    shared["Wm1_a"] = np.ascontiguousarray(Wm1_f).astype(np.float16)
    shared["bm1_a"] = colL(bm1_f, 32)
    shared["Wm2_a"] = np.ascontiguousarray(Wm2).astype(np.float16)
    shared["bm2_a"] = colL(bm2, 8)
    shared["lin_w_m"] = f(inputs["lin_w"])
    shared["lin_b_col"] = f(inputs["lin_b"]).reshape(27, 1)

    pos = f(inputs["pos_emb"])[0, :TN]                     # [192, 1024]
    pf = pos.T.reshape(8, 128, TN)                         # [dt, p, i]
    shared["pos_f"] = np.ascontiguousarray(
        np.broadcast_to(pf[:, :, None, :], (8, 128, 2, TN))
        .transpose(1, 0, 2, 3).reshape(128, 8 * NT)
    )

    states = f(inputs["states"])
    actions = f(inputs["actions"])[:, :, 0, :]
    rtgs = f(inputs["rtgs"])
    am = np.asarray(inputs["attention_mask"]).astype(bool)

    causal = np.tril(np.ones((TN, TN), bool))
    per_core = []
    for c in range(NCORES):
        sl = slice(2 * c, 2 * c + 2)
        d = dict(shared)
        d["states_f"] = np.ascontiguousarray(states[sl].reshape(2 * L, 91).T)
        d["act_f"] = np.ascontiguousarray(actions[sl].reshape(2 * L, 27).T)
        d["rtg_row"] = np.ascontiguousarray(rtgs[sl].reshape(1, 2 * L))
        mt = np.zeros((128, 4 * TN), np.float32)
        for s in range(2):
            m = np.repeat(am[2 * c + s], 3)                # [192]
            # A^T is indexed [j, i]; causal keeps j <= i (tril in [i, j])
            full = (m[:, None] & m[None, :] & causal.T).astype(np.float32)
            mt[:, (0 * 2 + s) * TN : (0 * 2 + s + 1) * TN] = full[0:128, :]
            mt[0:64, (1 * 2 + s) * TN : (1 * 2 + s + 1) * TN] = full[128:192, :]
        d["mask_t"] = mt.astype(np.float16)
        per_core.append(d)
    return per_core


def run(inputs, trace=False):
    nc = _get_nc()
    in_maps = host_prep(inputs)
    res = run_bass_kernel_spmd(
        nc, in_maps, list(range(NCORES)), trace=trace
    )
    outs = []
    for c in range(NCORES):
        o = res.results[c]["out_f"]                        # [27, 384]
        outs.append(o.T.reshape(2, TN, 27))
    full = np.concatenate(outs, axis=0).astype(np.float32)  # [16, 192, 27]
    return full, res


def kernel(**inputs) -> np.ndarray:
    out, _ = run(inputs, trace=False)
    return out
